# revision 12
# baseline (speedup 1.0000x reference)
"""Multi-head attention (B=2, S=4096, E=512, H=8) on 8 Trainium2 cores.

Sharding: one (batch, head-pair) unit per core - core c handles batch c//4
and heads 2*(c%4), 2*(c%4)+1.  Each core runs the full pipeline for its two
heads: QKV projection, flash-style attention (no S^2 materialization in
DRAM), and its partial output projection (Wo row-slice).  The host sums the
four partials per batch and adds the fused bias (bo + bv @ Wo).

v5 design (522us baseline -> this):
  - Host ships x/Wq/Wk/Wv in bf16: halves the input DMA (the phase-A wall).
    Each input streams in two priority-ordered halves on the sync queue
    (xk -> xq -> xv) so the projections chase the DMA; output tiles and the
    head1 partition shift go out via GpSimd's queue.
  - Phase A computes all of khT/qhT (bf16) and vh (fp16, with the wm/ones
    denominator column per head) chasing those DMAs.
  - Logits are emitted as adjacent row-tile pairs (lhsT base partitions
    0/64 auto-derive PE tile positions -> the two matmuls run concurrently,
    measured dt=3ns), four pairs per group to amortize PE weight-swap
    latency against the attnV stream.
  - exp splits per key-block pair across ScalarE table exp (20/32, fp16
    out) and VectorE Schraudolph bit-trick exp (12/32):
    fp16 = bitcast(int16(A*logit + B)), one tensor_scalar instruction,
    ~2% per-weight error that the softmax largely cancels.
  - attnV (fp16, K=128, M=65 with the ones column) accumulates both heads
    in two PSUM banks; it trails the logits/exp stream by LAG=8 pairs so
    the in-order PE queue neither waits on a fresh exp nor on the previous
    block's accumulator banks being released by the normalize.
  - The per-block tail is split and deferred: normalize (den broadcast +
    reciprocal_approx_fast + scale) is emitted at the next block's start,
    the Wo projection three groups later, so the PE work they inject never
    waits on the cross-engine chain and the HAM clock gate stays at 8/8.
"""

import math

import numpy as np
from contextlib import ExitStack

import ml_dtypes

import concourse.bass as bass
import concourse.bacc as bacc
import concourse.tile as tile
from concourse import mybir
from concourse.alu_op_type import AluOpType
from concourse.bass_utils import run_bass_kernel_spmd

F32 = mybir.dt.float32
F32R = mybir.dt.float32r
BF16 = mybir.dt.bfloat16
F16 = mybir.dt.float16
I16 = mybir.dt.int16

B = 2
S = 4096
E = 512
H = 8
D = 64
NCORES = 8
HPC = 2            # heads per core
DH = HPC * D       # 128
SQ = 512           # q-block (matmul moving free dim)
SKB = 128          # k-block (one partition tile)
ET = E // 128      # e-tiles in the contraction
LAG = 8            # attnV pipeline distance behind the logits/exp stream
GRP = 4            # logits pairs per emission group

# Schraudolph exp in fp16: exp(l/8) ~ bitcast_f16(int16(A*l + B)); the 1/8
# softmax scale folds into A.  B tuned numerically for min max-rel-err (~3%).
A_SCH = float((1 << 10) * math.log2(math.e) * 0.125)
B_SCH = 15315.75


def _exp_engine(kb: int, approx: bool) -> str:
    # GpSimd has no PSUM port, so exp splits between ScalarE (20/32 pairs)
    # and VectorE (12/32, Schraudolph), interleaved to avoid long ScalarE runs.
    if approx and kb % 8 in (1, 4, 6):
        return "dve"
    return "act"


_NC_CACHE = {}


def _build_kernel(ctx, tc, s, approx):
    nc = tc.nc
    nsq = s // SQ
    nsk = s // SKB
    AF = mybir.ActivationFunctionType

    xqT = nc.declare_dram_parameter("xqT", [E, s], BF16, isOutput=False)
    xkT = nc.declare_dram_parameter("xkT", [E, s], BF16, isOutput=False)
    xvT = nc.declare_dram_parameter("xvT", [E, s], BF16, isOutput=False)
    wq = nc.declare_dram_parameter("wq", [E, DH], BF16, isOutput=False)
    wk = nc.declare_dram_parameter("wk", [E, DH], BF16, isOutput=False)
    wv = nc.declare_dram_parameter("wv", [E, DH], BF16, isOutput=False)
    wo = nc.declare_dram_parameter("wo", [DH, E], F32, isOutput=False)
    bq = nc.declare_dram_parameter("bq", [DH], F32, isOutput=False)
    bk = nc.declare_dram_parameter("bk", [DH], F32, isOutput=False)
    wm = nc.declare_dram_parameter("wm", [s], F32, isOutput=False)
    out = nc.declare_dram_parameter("out", [E, s], F32, isOutput=True)

    const = ctx.enter_context(tc.tile_pool(name="const", bufs=1))
    res = ctx.enter_context(tc.tile_pool(name="res", bufs=1))

    # Input staging: two DMA halves per stream, priority-ordered so the
    # attention-critical data (kh, then qh block 0) lands first and the
    # projections can chase each half.
    sh = s // 2
    nkp = 4 if s >= 2048 else 2
    skp = s // nkp
    x_tiles = {}
    x_tiles["xk"] = tuple(res.tile([128, ET, skp], BF16, name=f"xk_{i}")
                          for i in range(nkp))
    for name in ("xq", "xv"):
        x_tiles[name] = (res.tile([128, ET, sh], BF16, name=f"{name}_a"),
                         res.tile([128, ET, sh], BF16, name=f"{name}_b"))
    wq_sb = const.tile([128, ET, DH], BF16)
    wk_sb = const.tile([128, ET, DH], BF16)
    wv_sb = const.tile([128, ET, DH], BF16)
    wo_sb = const.tile([128, E], F32R)
    bq_sb = const.tile([128, 1], F32)
    bk_sb = const.tile([128, 1], F32)
    wm_sb = const.tile([128, nsk], F32)

    def dma_half(name, dram, half):
        r = dram.rearrange("(t p) s -> p t s", p=128)
        nc.sync.dma_start(x_tiles[name][half],
                          r[:, :, half * sh:(half + 1) * sh])

    nc.sync.dma_start(wk_sb[:], wk.rearrange("(t p) d -> p t d", p=128))
    nc.sync.dma_start(bk_sb[:], bk.rearrange("(p o) -> p o", o=1))
    xk_r = xkT.rearrange("(t p) s -> p t s", p=128)
    for i in range(nkp):
        nc.sync.dma_start(x_tiles["xk"][i], xk_r[:, :, i * skp:(i + 1) * skp])
    nc.sync.dma_start(wq_sb[:], wq.rearrange("(t p) d -> p t d", p=128))
    nc.sync.dma_start(bq_sb[:], bq.rearrange("(p o) -> p o", o=1))
    dma_half("xq", xqT, 0)
    nc.sync.dma_start(wm_sb[:], wm.rearrange("(t p) -> p t", p=128))
    nc.sync.dma_start(wv_sb[:], wv.rearrange("(t p) d -> p t d", p=128))
    dma_half("xv", xvT, 0)
    dma_half("xv", xvT, 1)
    nc.sync.dma_start(wo_sb[:], wo[:, :].bitcast(F32R))
    # xq second half feeds q-blocks 4-7 only; lowest priority
    dma_half("xq", xqT, 1)

    # ones row (at partition D) used as lhsT of the K=1 broadcast matmul
    # (memset cannot emit f32r, so stage through an f32 tile)
    ones_f32 = const.tile([128, D], F32)
    nc.vector.memset(ones_f32[:], 1.0)
    ones_bc = const.tile([128, D], F32R)
    nc.vector.tensor_copy(ones_bc[:], ones_f32[:])
    # touch Exp once so the ACT table set loads during the phase-A DMA wait
    scratch = const.tile([1, 1], F32)
    nc.scalar.activation(scratch[:], ones_f32[0:1, 0:1], AF.Exp)

    # Resident K^T / Q^T (d-major, both heads stacked) and V (s-major, with
    # wm/ones column per head)
    khT = res.tile([128, s], BF16)
    qhT = res.tile([128, s], BF16)
    vh = res.tile([128, nsk, 2 * (D + 1)], F16)

    # wm/ones columns of vh (col 64 = head0, col 129 = head1)
    nc.vector.tensor_copy(vh[:, :, D], wm_sb[:, :])
    nc.vector.tensor_copy(vh[:, :, 2 * D + 1], wm_sb[:, :])

    lg_pool = ctx.enter_context(tc.tile_pool(name="lg", bufs=3, space="PSUM"))
    acc_pool = ctx.enter_context(tc.tile_pool(name="acc", bufs=2, space="PSUM"))
    ex_pool = ctx.enter_context(tc.tile_pool(name="expp", bufs=14))
    o_pool = ctx.enter_context(tc.tile_pool(name="o", bufs=2))
    sm_pool = ctx.enter_context(tc.tile_pool(name="sm", bufs=4))

    out_r = out.rearrange("(t p) s -> p t s", p=128)

    # ---- Phase A: K, Q, V projections (all of S), chasing the input DMA ----
    cs = min(SQ, sh)   # projection chunk (<= half size for small s)

    def proj_chunk(xname, w_sb, dst, b_sb, blk):
        ssl = slice(blk * cs, (blk + 1) * cs)
        psz = skp if xname == "xk" else sh
        half = (blk * cs) // psz
        hsl = slice(blk * cs - half * psz, (blk + 1) * cs - half * psz)
        x_t = x_tiles[xname][half]
        pk = lg_pool.tile([128, 2, SQ], F32, tag="lg", name="pkq")
        for et in range(ET):
            nc.tensor.matmul(
                pk[:, 0, 0:cs],
                lhsT=w_sb[:, et, :],
                rhs=x_t[:, et, hsl],
                start=(et == 0),
                stop=(et == ET - 1),
            )
        nc.vector.tensor_scalar_add(dst[:, ssl], pk[:, 0, 0:cs], b_sb[:, 0:1])

    def vproj_chunk(blk):
        for sub in range(SQ // SKB):
            s32 = blk * (SQ // SKB) + sub
            half = (s32 * SKB) // sh
            hsl = slice(s32 * SKB - half * sh, (s32 + 1) * SKB - half * sh)
            pv = lg_pool.tile([128, 2, SQ], F32, tag="lg", name="pv")
            for et in range(ET):
                nc.tensor.matmul(
                    pv[:, 0, 0:DH],
                    lhsT=x_tiles["xv"][half][:, et, hsl],
                    rhs=wv_sb[:, et, :],
                    start=(et == 0),
                    stop=(et == ET - 1),
                )
            wcol = wm_sb[:, s32:s32 + 1]
            nc.vector.tensor_scalar_mul(vh[:, s32, 0:D], pv[:, 0, 0:D], wcol)
            nc.vector.tensor_scalar_mul(
                vh[:, s32, D + 1:2 * D + 1], pv[:, 0, D:DH], wcol)

    nq_chunks = s // cs
    for blk in range(nq_chunks):
        proj_chunk("xk", wk_sb, khT, bk_sb, blk)
    # qh for attention block 0 only; the rest chases the xq/xv DMA inside
    # block 0's emission loop below
    for blk in range(SQ // cs):
        proj_chunk("xq", wq_sb, qhT, bq_sb, blk)

    # ---- Phase B: attention + deferred normalize/output projection ----
    def emit_logits_exp(sqi, kb):
        sqsl = slice(sqi * SQ, (sqi + 1) * SQ)
        kbsl = slice(kb * SKB, (kb + 1) * SKB)
        lgt = lg_pool.tile([128, 2, SQ], F32, tag="lg", name="lgt")
        # adjacent row-tile pair: base partitions 0/64 -> concurrent on PE
        nc.tensor.matmul(lgt[:, 0, :], lhsT=khT[0:D, kbsl], rhs=qhT[0:D, sqsl],
                         start=True, stop=True)
        nc.tensor.matmul(lgt[:, 1, :], lhsT=khT[D:DH, kbsl], rhs=qhT[D:DH, sqsl],
                         start=True, stop=True)
        ex = ex_pool.tile([128, 2, SQ], F16, tag="ex", name="ex")
        if _exp_engine(kb, approx) == "act":
            nc.scalar.activation(ex[:, :, :], lgt[:, :, :], AF.Exp, scale=0.125)
        else:
            nc.vector.tensor_scalar(
                ex[:, :, :].bitcast(I16), lgt[:, :, :],
                A_SCH, B_SCH, AluOpType.mult, AluOpType.add)
        return ex

    def emit_attnv(kb, ex, acc0, acc1):
        for h, acc in ((0, acc0), (1, acc1)):
            nc.tensor.matmul(
                acc[0:D + 1, :],
                lhsT=vh[:, kb, h * (D + 1):(h + 1) * (D + 1)],
                rhs=ex[:, h, :],
                start=(kb == 0),
                stop=(kb == nsk - 1),
            )

    def emit_tail_a(acc0, acc1):
        """Normalize: unnormalized head outputs (PSUM) -> ostage (SBUF)."""
        ostage = o_pool.tile([128, SQ], F32R, tag="onorm", name="ostage")
        tmp1 = o_pool.tile([64, SQ], F32R, tag="tmp1", name="tmp1")
        bct = lg_pool.tile([128, 2, SQ], F32, tag="lg", name="bct")
        for h, acc in ((0, acc0), (1, acc1)):
            # denominator (row D of acc) to SBUF as f32r, broadcast across the
            # D partitions with a K=1 matmul, then fast reciprocal on 64 lanes
            den = sm_pool.tile([128, SQ], F32R, tag="den", name="den")
            nc.vector.tensor_copy(den[D:D + 1, :], acc[D:D + 1, :])
            nc.tensor.matmul(
                bct[0:D, h, :],
                lhsT=ones_bc[D:D + 1, :],
                rhs=den[D:D + 1, :],
                start=True, stop=True)
            bc = sm_pool.tile([64, SQ], F32, tag="bc", name="bc")
            nc.vector.reciprocal_approx_fast(bc[:], bct[0:D, h, :])
            if h == 0:
                nc.vector.tensor_mul(ostage[0:D, :], acc[0:D, :], bc[:])
            else:
                # DVE lanes can't shift partitions; route head1 through SBUF DMA
                nc.vector.tensor_mul(tmp1[:], acc[0:D, :], bc[:])
                nc.gpsimd.dma_start(ostage[D:DH, :], tmp1[:])
        return ostage

    def emit_tail_b(sqi, ostage):
        """Wo projection of the normalized block + output DMA."""
        sqsl = slice(sqi * SQ, (sqi + 1) * SQ)
        for m0 in range(0, ET, 2):
            pp = lg_pool.tile([128, 2, SQ], F32, tag="lg", name="pp")
            for j, m in enumerate((m0, m0 + 1)):
                nc.tensor.matmul(
                    pp[:, j, :],
                    lhsT=wo_sb[:, m * 128:(m + 1) * 128],
                    rhs=ostage[:],
                    start=True, stop=True)
                ot = o_pool.tile([128, SQ], F32, tag="ot", name="ot")
                nc.vector.tensor_copy(ot[:], pp[:, j, :])
                nc.gpsimd.dma_start(out_r[:, m, sqsl], ot[:])

    # attnV trails the logits/exp stream by LAG pairs (in groups of GRP to
    # amortize PE weight-swap latency) so the in-order PE queue waits neither
    # on a freshly-issued exp nor on the deferred tail of the previous block.
    prev = None
    for sqi in range(nsq):
        acc0 = acc_pool.tile([128, SQ], F32, tag="acc", name="acc0")
        acc1 = acc_pool.tile([128, SQ], F32, tag="acc", name="acc1")
        exs = {}
        for t0 in range(0, nsk + LAG, GRP):
            if t0 == 0 and prev is not None:
                prev_ostage = emit_tail_a(prev[0], prev[1])
            for t in range(t0, t0 + GRP):
                if t < nsk:
                    exs[t] = emit_logits_exp(sqi, t)
            if sqi == 0:
                # remaining projections chase their DMAs interleaved with
                # block 0's attention stream
                g = t0 // GRP
                qblk0 = (g + 1) * (SQ // cs)
                for blk in range(qblk0, min(qblk0 + SQ // cs, nq_chunks)):
                    proj_chunk("xq", wq_sb, qhT, bq_sb, blk)
                if g < nsq:
                    vproj_chunk(g)
            if t0 == 3 * GRP and prev is not None:
                emit_tail_b(sqi - 1, prev_ostage)
            for t in range(t0, t0 + GRP):
                kb = t - LAG
                if 0 <= kb < nsk:
                    emit_attnv(kb, exs.pop(kb), acc0, acc1)
        prev = (acc0, acc1)
    ostage = emit_tail_a(prev[0], prev[1])
    emit_tail_b(nsq - 1, ostage)


def build_nc(s=S, approx=True):
    key = (s, approx)
    if key in _NC_CACHE:
        return _NC_CACHE[key]
    nc = bacc.Bacc("TRN2", target_bir_lowering=False, debug=False)
    with tile.TileContext(nc) as tc:
        with ExitStack() as ctx:
            _build_kernel(ctx, tc, s, approx)
    nc.compile()
    _NC_CACHE[key] = nc
    return nc


def make_in_maps(q, k, v, mask, Wq, bq, Wk, bk, Wv, bv, Wo, bo):
    bf16 = ml_dtypes.bfloat16
    q = np.asarray(q, np.float32)
    k = np.asarray(k, np.float32)
    v = np.asarray(v, np.float32)
    mask = np.asarray(mask, np.float32)
    Wq16 = np.asarray(Wq, np.float32).astype(bf16)
    Wk16 = np.asarray(Wk, np.float32).astype(bf16)
    Wv16 = np.asarray(Wv, np.float32).astype(bf16)
    Wo = np.asarray(Wo, np.float32)
    bq = np.asarray(bq, np.float32)
    bk = np.asarray(bk, np.float32)

    xT = {}
    wmb = {}
    for b in range(q.shape[0]):
        xT[("q", b)] = np.ascontiguousarray(q[b].T.astype(bf16))
        xT[("k", b)] = np.ascontiguousarray(k[b].T.astype(bf16))
        xT[("v", b)] = np.ascontiguousarray(v[b].T.astype(bf16))
        # additive mask -> exact multiplicative per-key weight
        wmb[b] = np.exp(np.float32(-1e9) * mask[b, 0, 0, :]).astype(np.float32)

    in_maps = []
    for c in range(NCORES):
        b = c // (NCORES // B)
        p = c % (NCORES // B)
        hsl = slice(p * DH, (p + 1) * DH)
        in_maps.append({
            "xqT": xT[("q", b)],
            "xkT": xT[("k", b)],
            "xvT": xT[("v", b)],
            "wq": np.ascontiguousarray(Wq16[:, hsl]),
            "wk": np.ascontiguousarray(Wk16[:, hsl]),
            "wv": np.ascontiguousarray(Wv16[:, hsl]),
            "wo": np.ascontiguousarray(Wo[hsl, :]),
            "bq": np.ascontiguousarray(bq[hsl]),
            "bk": np.ascontiguousarray(bk[hsl]),
            "wm": wmb[b],
        })
    return in_maps


def gather(results, bv, bo, Wo):
    bias_total = (np.asarray(bv, np.float32) @ np.asarray(Wo, np.float32)
                  + np.asarray(bo, np.float32))
    cpb = NCORES // B
    full = np.empty((B, S, E), np.float32)
    for b in range(B):
        acc = results[b * cpb]["out"].astype(np.float32, copy=True)
        for c in range(b * cpb + 1, (b + 1) * cpb):
            acc += results[c]["out"]
        full[b] = acc.T + bias_total
    return full


def run(trace=False, approx=True, **inputs):
    nc = build_nc(S, approx)
    in_maps = make_in_maps(
        inputs["q"], inputs["k"], inputs["v"], inputs["mask"],
        inputs["Wq"], inputs["bq"], inputs["Wk"], inputs["bk"],
        inputs["Wv"], inputs["bv"], inputs["Wo"], inputs["bo"],
    )
    res = run_bass_kernel_spmd(nc, in_maps, list(range(NCORES)), trace=trace)
    out = gather(res.results, inputs["bv"], inputs["bo"], inputs["Wo"])
    return out, res


def kernel(**inputs):
    out, _ = run(trace=False, **inputs)
    return out


# revision 13
# speedup vs baseline: 1.0009x; 1.0009x over previous
"""Multi-head attention (B=2, S=4096, E=512, H=8) on 8 Trainium2 cores.

Sharding: one (batch, head-pair) unit per core - core c handles batch c//4
and heads 2*(c%4), 2*(c%4)+1.  Each core runs the full pipeline for its two
heads: QKV projection, flash-style attention (no S^2 materialization in
DRAM), and its partial output projection (Wo row-slice).  The host sums the
four partials per batch and adds the fused bias (bo + bv @ Wo).

v5 design (522us baseline -> this):
  - Host ships x/Wq/Wk/Wv in bf16: halves the input DMA (the phase-A wall).
    Each input streams in two priority-ordered halves on the sync queue
    (xk -> xq -> xv) so the projections chase the DMA; output tiles and the
    head1 partition shift go out via GpSimd's queue.
  - Phase A computes all of khT/qhT (bf16) and vh (fp16, with the wm/ones
    denominator column per head) chasing those DMAs.
  - Logits are emitted as adjacent row-tile pairs (lhsT base partitions
    0/64 auto-derive PE tile positions -> the two matmuls run concurrently,
    measured dt=3ns), four pairs per group to amortize PE weight-swap
    latency against the attnV stream.
  - exp splits per key-block pair across ScalarE table exp (20/32, fp16
    out) and VectorE Schraudolph bit-trick exp (12/32):
    fp16 = bitcast(int16(A*logit + B)), one tensor_scalar instruction,
    ~2% per-weight error that the softmax largely cancels.
  - attnV (fp16, K=128, M=65 with the ones column) accumulates both heads
    in two PSUM banks; it trails the logits/exp stream by LAG=8 pairs so
    the in-order PE queue neither waits on a fresh exp nor on the previous
    block's accumulator banks being released by the normalize.
  - The per-block tail is split and deferred: normalize (den broadcast +
    reciprocal_approx_fast + scale) is emitted at the next block's start,
    the Wo projection three groups later, so the PE work they inject never
    waits on the cross-engine chain and the HAM clock gate stays at 8/8.
"""

import math

import numpy as np
from contextlib import ExitStack

import ml_dtypes

import concourse.bass as bass
import concourse.bacc as bacc
import concourse.tile as tile
from concourse import mybir
from concourse.alu_op_type import AluOpType
from concourse.bass_utils import run_bass_kernel_spmd

F32 = mybir.dt.float32
F32R = mybir.dt.float32r
BF16 = mybir.dt.bfloat16
F16 = mybir.dt.float16
I16 = mybir.dt.int16

B = 2
S = 4096
E = 512
H = 8
D = 64
NCORES = 8
HPC = 2            # heads per core
DH = HPC * D       # 128
SQ = 512           # q-block (matmul moving free dim)
SKB = 128          # k-block (one partition tile)
ET = E // 128      # e-tiles in the contraction
LAG = 8            # attnV pipeline distance behind the logits/exp stream
GRP = 4            # logits pairs per emission group

# Schraudolph exp in fp16: exp(l/8) ~ bitcast_f16(int16(A*l + B)); the 1/8
# softmax scale folds into A.  B tuned numerically for min max-rel-err (~3%).
A_SCH = float((1 << 10) * math.log2(math.e) * 0.125)
B_SCH = 15315.75


def _exp_engine(kb: int, approx: bool) -> str:
    # GpSimd has no PSUM port, so exp splits between ScalarE (20/32 pairs)
    # and VectorE (12/32, Schraudolph), interleaved to avoid long ScalarE runs.
    if approx and kb % 8 in (1, 4, 6):
        return "dve"
    return "act"


_NC_CACHE = {}


def _build_kernel(ctx, tc, s, approx):
    nc = tc.nc
    nsq = s // SQ
    nsk = s // SKB
    AF = mybir.ActivationFunctionType

    xqT = nc.declare_dram_parameter("xqT", [E, s], BF16, isOutput=False)
    xkT = nc.declare_dram_parameter("xkT", [E, s], BF16, isOutput=False)
    xvT = nc.declare_dram_parameter("xvT", [E, s], BF16, isOutput=False)
    wq = nc.declare_dram_parameter("wq", [E, DH], BF16, isOutput=False)
    wk = nc.declare_dram_parameter("wk", [E, DH], BF16, isOutput=False)
    wv = nc.declare_dram_parameter("wv", [E, DH], BF16, isOutput=False)
    wo = nc.declare_dram_parameter("wo", [DH, E], F32, isOutput=False)
    bq = nc.declare_dram_parameter("bq", [DH], F32, isOutput=False)
    bk = nc.declare_dram_parameter("bk", [DH], F32, isOutput=False)
    wm = nc.declare_dram_parameter("wm", [s], F32, isOutput=False)
    out = nc.declare_dram_parameter("out", [E, s], F32, isOutput=True)

    const = ctx.enter_context(tc.tile_pool(name="const", bufs=1))
    res = ctx.enter_context(tc.tile_pool(name="res", bufs=1))

    # Input staging: two DMA halves per stream, priority-ordered so the
    # attention-critical data (kh, then qh block 0) lands first and the
    # projections can chase each half.
    sh = s // 2
    nkp = 4 if s >= 2048 else 2
    skp = s // nkp
    x_tiles = {}
    x_tiles["xk"] = tuple(res.tile([128, ET, skp], BF16, name=f"xk_{i}")
                          for i in range(nkp))
    for name in ("xq", "xv"):
        x_tiles[name] = (res.tile([128, ET, sh], BF16, name=f"{name}_a"),
                         res.tile([128, ET, sh], BF16, name=f"{name}_b"))
    wq_sb = const.tile([128, ET, DH], BF16)
    wk_sb = const.tile([128, ET, DH], BF16)
    wv_sb = const.tile([128, ET, DH], BF16)
    wo_sb = const.tile([128, E], F32R)
    bq_sb = const.tile([128, 1], F32)
    bk_sb = const.tile([128, 1], F32)
    wm_sb = const.tile([128, nsk], F32)

    def dma_half(name, dram, half):
        r = dram.rearrange("(t p) s -> p t s", p=128)
        nc.sync.dma_start(x_tiles[name][half],
                          r[:, :, half * sh:(half + 1) * sh])

    nc.sync.dma_start(wk_sb[:], wk.rearrange("(t p) d -> p t d", p=128))
    nc.sync.dma_start(bk_sb[:], bk.rearrange("(p o) -> p o", o=1))
    xk_r = xkT.rearrange("(t p) s -> p t s", p=128)
    for i in range(nkp):
        nc.sync.dma_start(x_tiles["xk"][i], xk_r[:, :, i * skp:(i + 1) * skp])
    nc.sync.dma_start(wq_sb[:], wq.rearrange("(t p) d -> p t d", p=128))
    nc.sync.dma_start(bq_sb[:], bq.rearrange("(p o) -> p o", o=1))
    dma_half("xq", xqT, 0)
    nc.sync.dma_start(wm_sb[:], wm.rearrange("(t p) -> p t", p=128))
    nc.sync.dma_start(wv_sb[:], wv.rearrange("(t p) d -> p t d", p=128))
    dma_half("xv", xvT, 0)
    dma_half("xv", xvT, 1)
    nc.sync.dma_start(wo_sb[:], wo[:, :].bitcast(F32R))
    # xq second half feeds q-blocks 4-7 only; lowest priority
    dma_half("xq", xqT, 1)

    # ones row (at partition D) used as lhsT of the K=1 broadcast matmul
    # (memset cannot emit f32r, so stage through an f32 tile)
    ones_f32 = const.tile([128, D], F32)
    nc.vector.memset(ones_f32[:], 1.0)
    ones_bc = const.tile([128, D], F32R)
    nc.vector.tensor_copy(ones_bc[:], ones_f32[:])
    # touch Exp once so the ACT table set loads during the phase-A DMA wait
    scratch = const.tile([1, 1], F32)
    nc.scalar.activation(scratch[:], ones_f32[0:1, 0:1], AF.Exp)

    # Resident K^T / Q^T (d-major, both heads stacked) and V (s-major, with
    # wm/ones column per head)
    khT = res.tile([128, s], BF16)
    qhT = res.tile([128, s], BF16)
    vh = res.tile([128, nsk, 2 * (D + 1)], F16)

    # wm/ones columns of vh (col 64 = head0, col 129 = head1)
    nc.vector.tensor_copy(vh[:, :, D], wm_sb[:, :])
    nc.vector.tensor_copy(vh[:, :, 2 * D + 1], wm_sb[:, :])

    lg_pool = ctx.enter_context(tc.tile_pool(name="lg", bufs=3, space="PSUM"))
    acc_pool = ctx.enter_context(tc.tile_pool(name="acc", bufs=2, space="PSUM"))
    ex_pool = ctx.enter_context(tc.tile_pool(name="expp", bufs=14))
    o_pool = ctx.enter_context(tc.tile_pool(name="o", bufs=2))
    sm_pool = ctx.enter_context(tc.tile_pool(name="sm", bufs=4))

    out_r = out.rearrange("(t p) s -> p t s", p=128)

    # ---- Phase A: K, Q, V projections (all of S), chasing the input DMA ----
    cs = min(SQ, sh)   # projection chunk (<= half size for small s)

    def proj_chunk(xname, w_sb, dst, b_sb, blk):
        ssl = slice(blk * cs, (blk + 1) * cs)
        psz = skp if xname == "xk" else sh
        half = (blk * cs) // psz
        hsl = slice(blk * cs - half * psz, (blk + 1) * cs - half * psz)
        x_t = x_tiles[xname][half]
        pk = lg_pool.tile([128, 2, SQ], F32, tag="lg", name="pkq")
        for et in range(ET):
            nc.tensor.matmul(
                pk[:, 0, 0:cs],
                lhsT=w_sb[:, et, :],
                rhs=x_t[:, et, hsl],
                start=(et == 0),
                stop=(et == ET - 1),
            )
        nc.vector.tensor_scalar_add(dst[:, ssl], pk[:, 0, 0:cs], b_sb[:, 0:1])

    def vproj_chunk(blk):
        for sub in range(SQ // SKB):
            s32 = blk * (SQ // SKB) + sub
            half = (s32 * SKB) // sh
            hsl = slice(s32 * SKB - half * sh, (s32 + 1) * SKB - half * sh)
            pv = lg_pool.tile([128, 2, SQ], F32, tag="lg", name="pv")
            for et in range(ET):
                nc.tensor.matmul(
                    pv[:, 0, 0:DH],
                    lhsT=x_tiles["xv"][half][:, et, hsl],
                    rhs=wv_sb[:, et, :],
                    start=(et == 0),
                    stop=(et == ET - 1),
                )
            wcol = wm_sb[:, s32:s32 + 1]
            nc.vector.tensor_scalar_mul(vh[:, s32, 0:D], pv[:, 0, 0:D], wcol)
            nc.vector.tensor_scalar_mul(
                vh[:, s32, D + 1:2 * D + 1], pv[:, 0, D:DH], wcol)

    nq_chunks = s // cs
    for blk in range(nq_chunks):
        proj_chunk("xk", wk_sb, khT, bk_sb, blk)
    # qh for attention block 0 only; the rest chases the xq/xv DMA inside
    # block 0's emission loop below
    for blk in range(SQ // cs):
        proj_chunk("xq", wq_sb, qhT, bq_sb, blk)

    # ---- Phase B: attention + deferred normalize/output projection ----
    def emit_logits_exp(sqi, kb):
        sqsl = slice(sqi * SQ, (sqi + 1) * SQ)
        kbsl = slice(kb * SKB, (kb + 1) * SKB)
        lgt = lg_pool.tile([128, 2, SQ], F32, tag="lg", name="lgt")
        # adjacent row-tile pair: base partitions 0/64 -> concurrent on PE
        nc.tensor.matmul(lgt[:, 0, :], lhsT=khT[0:D, kbsl], rhs=qhT[0:D, sqsl],
                         start=True, stop=True)
        nc.tensor.matmul(lgt[:, 1, :], lhsT=khT[D:DH, kbsl], rhs=qhT[D:DH, sqsl],
                         start=True, stop=True)
        ex = ex_pool.tile([128, 2, SQ], F16, tag="ex", name="ex")
        if _exp_engine(kb, approx) == "act":
            nc.scalar.activation(ex[:, :, :], lgt[:, :, :], AF.Exp, scale=0.125)
        else:
            nc.vector.tensor_scalar(
                ex[:, :, :].bitcast(I16), lgt[:, :, :],
                A_SCH, B_SCH, AluOpType.mult, AluOpType.add)
        return ex

    def emit_attnv(kb, ex, acc0, acc1):
        for h, acc in ((0, acc0), (1, acc1)):
            nc.tensor.matmul(
                acc[0:D + 1, :],
                lhsT=vh[:, kb, h * (D + 1):(h + 1) * (D + 1)],
                rhs=ex[:, h, :],
                start=(kb == 0),
                stop=(kb == nsk - 1),
            )

    def emit_tail_a(acc0, acc1):
        """Normalize: unnormalized head outputs (PSUM) -> ostage (SBUF)."""
        ostage = o_pool.tile([128, SQ], F32R, tag="onorm", name="ostage")
        tmp1 = o_pool.tile([64, SQ], F32R, tag="tmp1", name="tmp1")
        for h, acc in ((0, acc0), (1, acc1)):
            # denominator (row D of acc) to SBUF as f32r, broadcast across the
            # D partitions with a K=1 matmul, then fast reciprocal on 64 lanes
            den = sm_pool.tile([128, SQ], F32R, tag="den", name="den")
            nc.vector.tensor_copy(den[D:D + 1, :], acc[D:D + 1, :])
            bct = lg_pool.tile([128, 2, SQ], F32, tag="lg", name="bct")
            nc.tensor.matmul(
                bct[0:D, 0, :],
                lhsT=ones_bc[D:D + 1, :],
                rhs=den[D:D + 1, :],
                start=True, stop=True)
            bc = sm_pool.tile([64, SQ], F32, tag="bc", name="bc")
            nc.vector.reciprocal_approx_fast(bc[:], bct[0:D, 0, :])
            if h == 0:
                nc.vector.tensor_mul(ostage[0:D, :], acc[0:D, :], bc[:])
            else:
                # DVE lanes can't shift partitions; route head1 through SBUF DMA
                nc.vector.tensor_mul(tmp1[:], acc[0:D, :], bc[:])
                nc.gpsimd.dma_start(ostage[D:DH, :], tmp1[:])
        return ostage

    def emit_tail_b(sqi, ostage):
        """Wo projection of the normalized block + output DMA."""
        sqsl = slice(sqi * SQ, (sqi + 1) * SQ)
        for m in range(ET):
            pp = lg_pool.tile([128, 2, SQ], F32, tag="lg", name="pp")
            nc.tensor.matmul(
                pp[:, 0, :],
                lhsT=wo_sb[:, m * 128:(m + 1) * 128],
                rhs=ostage[:],
                start=True, stop=True)
            ot = o_pool.tile([128, SQ], F32, tag="ot", name="ot")
            nc.vector.tensor_copy(ot[:], pp[:, 0, :])
            nc.gpsimd.dma_start(out_r[:, m, sqsl], ot[:])

    # attnV trails the logits/exp stream by LAG pairs (in groups of GRP to
    # amortize PE weight-swap latency) so the in-order PE queue waits neither
    # on a freshly-issued exp nor on the deferred tail of the previous block.
    prev = None
    for sqi in range(nsq):
        acc0 = acc_pool.tile([128, SQ], F32, tag="acc", name="acc0")
        acc1 = acc_pool.tile([128, SQ], F32, tag="acc", name="acc1")
        exs = {}
        for t0 in range(0, nsk + LAG, GRP):
            if t0 == 0 and prev is not None:
                prev_ostage = emit_tail_a(prev[0], prev[1])
            for t in range(t0, t0 + GRP):
                if t < nsk:
                    exs[t] = emit_logits_exp(sqi, t)
            if sqi == 0:
                # remaining projections chase their DMAs interleaved with
                # block 0's attention stream
                g = t0 // GRP
                qblk0 = (g + 1) * (SQ // cs)
                for blk in range(qblk0, min(qblk0 + SQ // cs, nq_chunks)):
                    proj_chunk("xq", wq_sb, qhT, bq_sb, blk)
                if g < nsq:
                    vproj_chunk(g)
            if t0 == 3 * GRP and prev is not None:
                emit_tail_b(sqi - 1, prev_ostage)
            for t in range(t0, t0 + GRP):
                kb = t - LAG
                if 0 <= kb < nsk:
                    emit_attnv(kb, exs.pop(kb), acc0, acc1)
        prev = (acc0, acc1)
    ostage = emit_tail_a(prev[0], prev[1])
    emit_tail_b(nsq - 1, ostage)


def build_nc(s=S, approx=True):
    key = (s, approx)
    if key in _NC_CACHE:
        return _NC_CACHE[key]
    nc = bacc.Bacc("TRN2", target_bir_lowering=False, debug=False)
    with tile.TileContext(nc) as tc:
        with ExitStack() as ctx:
            _build_kernel(ctx, tc, s, approx)
    nc.compile()
    _NC_CACHE[key] = nc
    return nc


def make_in_maps(q, k, v, mask, Wq, bq, Wk, bk, Wv, bv, Wo, bo):
    bf16 = ml_dtypes.bfloat16
    q = np.asarray(q, np.float32)
    k = np.asarray(k, np.float32)
    v = np.asarray(v, np.float32)
    mask = np.asarray(mask, np.float32)
    Wq16 = np.asarray(Wq, np.float32).astype(bf16)
    Wk16 = np.asarray(Wk, np.float32).astype(bf16)
    Wv16 = np.asarray(Wv, np.float32).astype(bf16)
    Wo = np.asarray(Wo, np.float32)
    bq = np.asarray(bq, np.float32)
    bk = np.asarray(bk, np.float32)

    xT = {}
    wmb = {}
    for b in range(q.shape[0]):
        xT[("q", b)] = np.ascontiguousarray(q[b].T.astype(bf16))
        xT[("k", b)] = np.ascontiguousarray(k[b].T.astype(bf16))
        xT[("v", b)] = np.ascontiguousarray(v[b].T.astype(bf16))
        # additive mask -> exact multiplicative per-key weight
        wmb[b] = np.exp(np.float32(-1e9) * mask[b, 0, 0, :]).astype(np.float32)

    in_maps = []
    for c in range(NCORES):
        b = c // (NCORES // B)
        p = c % (NCORES // B)
        hsl = slice(p * DH, (p + 1) * DH)
        in_maps.append({
            "xqT": xT[("q", b)],
            "xkT": xT[("k", b)],
            "xvT": xT[("v", b)],
            "wq": np.ascontiguousarray(Wq16[:, hsl]),
            "wk": np.ascontiguousarray(Wk16[:, hsl]),
            "wv": np.ascontiguousarray(Wv16[:, hsl]),
            "wo": np.ascontiguousarray(Wo[hsl, :]),
            "bq": np.ascontiguousarray(bq[hsl]),
            "bk": np.ascontiguousarray(bk[hsl]),
            "wm": wmb[b],
        })
    return in_maps


def gather(results, bv, bo, Wo):
    bias_total = (np.asarray(bv, np.float32) @ np.asarray(Wo, np.float32)
                  + np.asarray(bo, np.float32))
    cpb = NCORES // B
    full = np.empty((B, S, E), np.float32)
    for b in range(B):
        acc = results[b * cpb]["out"].astype(np.float32, copy=True)
        for c in range(b * cpb + 1, (b + 1) * cpb):
            acc += results[c]["out"]
        full[b] = acc.T + bias_total
    return full


def run(trace=False, approx=True, **inputs):
    nc = build_nc(S, approx)
    in_maps = make_in_maps(
        inputs["q"], inputs["k"], inputs["v"], inputs["mask"],
        inputs["Wq"], inputs["bq"], inputs["Wk"], inputs["bk"],
        inputs["Wv"], inputs["bv"], inputs["Wo"], inputs["bo"],
    )
    res = run_bass_kernel_spmd(nc, in_maps, list(range(NCORES)), trace=trace)
    out = gather(res.results, inputs["bv"], inputs["bo"], inputs["Wo"])
    return out, res


def kernel(**inputs):
    out, _ = run(trace=False, **inputs)
    return out
